# revision 3
# baseline (speedup 1.0000x reference)
"""Trainium2 Bass kernel for nn_EnhanceDiversityFeatureExtracition.

Computes  loss = mean((x-y)^2) + ALPHA * diversity_reg(conv_w)
where diversity_reg builds a 64x64 Gram matrix of the F=64 slices
conv_w[:, :, i, :] (each flattened to a 786432-vector), normalizes it to
cosine similarities, and sums the entries with tau < sim <= 1 off the
diagonal.

Distribution (8 NeuronCores, SPMD):
  - x_batch / y_batch sharded on batch dim: 256 rows per core.
  - conv_w viewed as A = conv_w.reshape(262144, 192)  (row m = (o,c),
    col = f*3+k).  gram[i,j] = sum_m sum_k A[m,3i+k]*A[m,3j+k], so A is
    sharded along the 262144-row reduction axis: 32768 rows per core.
  - Each core returns the partial C = A_shard^T A_shard (upper 128x192
    plus the lower-right 64x64 block; the lower-left comes from symmetry)
    and per-partition partial sums of (x-y)^2; the host sums partials,
    extracts gram[i,j] = sum_k C[3i+k,3j+k] and applies the tiny 64x64
    masked-similarity epilogue.

On-core dataflow (v3, tuned from traces):
  - A is streamed via SWDGE (gpsimd) DMAs that cast fp32 -> bf16 in the
    DMA datapath: HBM reads stay fp32 (the roofline term), SBUF holds
    bf16.  One strictly-FIFO ring => blocks land in program order.
    24KB-per-partition descriptors on the big blocks.
  - bf16 matmuls need no fp32r 256-wide moving trick: per 128-row strip
    one [128c x 128s x 192m] matmul (C rows 0:128) plus one
    [128c x 64s x 64m] matmul (C rows/cols 128:192) ~= 130ns/strip --
    ~2x headroom over the DMA feed rate, so the PE never becomes the
    critical path even when HAM-throttled cold, and supply jitter is
    absorbed.
  - x/y stream as 4 chunk-pairs of [128 x 2048] fp32 (1MB, 8KB
    descriptors) on the two HWDGE rings (Sync / Activation), co-draining
    with the A stream via the SDMA engines' per-packet round-robin.
  - MSE: DVE computes d = x-y, ACT computes Square(d) with per-chunk
    per-partition accumulate; ACTs are queued after the Activation
    ring's DMA issues so they never stall descriptor generation.
"""

import numpy as np

import concourse.bass as bass
import concourse.mybir as mybir
from concourse import bacc, tile
from concourse.bass_utils import run_bass_kernel_spmd

N_CORES = 8
B, D = 2048, 4096            # x_batch / y_batch
M, G = 262144, 192           # conv_w as (M, G); G = F*KW
F, KW = 64, 3
ROWS = B // N_CORES          # 256 batch rows per core
MC = M // N_CORES            # 32768 reduction rows per core
NSTRIP = MC // 128           # 256 strips of 128 rows per core

# A-block plan (strips per block); small blocks last shrink the tail.
A_PLAN = [32, 32, 32, 32, 32, 32, 32, 16, 10, 6]
assert sum(A_PLAN) == NSTRIP

NXY = 4                      # x/y chunk-pairs per core
XYW = (ROWS * D) // (128 * NXY)   # 2048 floats per partition per chunk

ALPHA = 0.0005
TAU = 0.2

_prog = None


def _build() -> bass.Bass:
    nc = bacc.Bacc(None, target_bir_lowering=False)
    f32 = mybir.dt.float32
    bf16 = mybir.dt.bfloat16

    xs = nc.dram_tensor("xs", [ROWS, D], f32, kind="ExternalInput")
    ys = nc.dram_tensor("ys", [ROWS, D], f32, kind="ExternalInput")
    aw = nc.dram_tensor("aw", [MC, G], f32, kind="ExternalInput")
    c1_part = nc.dram_tensor("c1_part", [128, G], f32, kind="ExternalOutput")
    c2_part = nc.dram_tensor("c2_part", [F, F], f32, kind="ExternalOutput")
    sse_part = nc.dram_tensor("sse_part", [128, NXY], f32, kind="ExternalOutput")

    MAXS = max(A_PLAN)

    with tile.TileContext(nc) as tc:
        with (
            tc.tile_pool(name="apool", bufs=6) as apool,
            tc.tile_pool(name="xpool", bufs=4) as xpool,
            tc.tile_pool(name="ypool", bufs=4) as ypool,
            tc.tile_pool(name="dpool", bufs=2) as dpool,
            tc.tile_pool(name="qpool", bufs=2) as qpool,
            tc.tile_pool(name="opool", bufs=1) as opool,
            tc.tile_pool(name="psum", bufs=1, space=bass.MemorySpace.PSUM) as psum,
        ):
            cps1 = psum.tile([128, G], f32, tag="cps1")
            cps2 = psum.tile([F, F], f32, tag="cps2")
            acc = opool.tile([128, NXY], f32)

            xv = xs[:].rearrange("(p t) d -> p (t d)", p=128)
            yv = ys[:].rearrange("(p t) d -> p (t d)", p=128)

            acts = []        # deferred (dtile, chunk-idx) Square-accums

            def emit_xy(c):
                xt = xpool.tile([128, XYW], f32)
                nc.sync.dma_start(xt[:], xv[:, c * XYW:(c + 1) * XYW])
                yt = ypool.tile([128, XYW], f32)
                nc.scalar.dma_start(yt[:], yv[:, c * XYW:(c + 1) * XYW])
                dtile = dpool.tile([128, XYW], f32)
                nc.vector.tensor_sub(dtile[:], xt[:], yt[:])
                acts.append((dtile, c))

            ti = 0           # global strip counter
            row0 = 0
            xy_next = 0
            for bi, ns in enumerate(A_PLAN):
                at = apool.tile([128, MAXS * G], bf16)
                src = aw[row0:row0 + 128 * ns].rearrange(
                    "(p t) g -> p (t g)", p=128)
                nc.gpsimd.dma_start(at[:, :ns * G], src)   # fp32->bf16 cast
                row0 += 128 * ns

                for t in range(ns):
                    w1 = at[:, t * G:t * G + 128]
                    rhs1 = at[:, t * G:t * G + G]
                    w2 = at[:, t * G + 128:t * G + G]
                    nc.tensor.matmul(
                        cps1[:], w1, rhs1,
                        start=(ti == 0), stop=(ti == NSTRIP - 1),
                    )
                    nc.tensor.matmul(
                        cps2[:], w2, w2,
                        start=(ti == 0), stop=(ti == NSTRIP - 1),
                    )
                    ti += 1

                if bi in (1, 3, 5, 7) and xy_next < NXY:
                    emit_xy(xy_next)
                    xy_next += 1
            while xy_next < NXY:
                emit_xy(xy_next)
                xy_next += 1

            # C partials out (ready once the last A block's matmuls stop)
            o1 = opool.tile([128, G], f32, tag="o1")
            nc.vector.tensor_copy(o1[:], cps1[:])
            nc.sync.dma_start(c1_part[:], o1[:])
            o2 = opool.tile([F, F], f32, tag="o2")
            nc.vector.tensor_copy(o2[:], cps2[:])
            nc.sync.dma_start(c2_part[:], o2[:])

            # deferred MSE squares: queued on ACT after all its DMA issues
            for dtile, c in acts:
                qtile = qpool.tile([128, XYW], f32)
                nc.scalar.activation(
                    qtile[:], dtile[:],
                    mybir.ActivationFunctionType.Square,
                    accum_out=acc[:, c:c + 1],
                )
            nc.scalar.dma_start(sse_part[:], acc[:])

    nc.finalize()
    return nc


def _get_prog() -> bass.Bass:
    global _prog
    if _prog is None:
        _prog = _build()
    return _prog


def _epilogue(C: np.ndarray, sse: float) -> np.ndarray:
    # gram[i,j] = sum_k C[3i+k, 3j+k]
    gram = np.einsum("ikjl,kl->ij", C.reshape(F, KW, F, KW), np.eye(KW))
    norms = np.sqrt(np.diag(gram))
    sim = gram / np.outer(norms, norms)
    mask = (sim > TAU) & (sim <= 1.0) & (~np.eye(F, dtype=bool))
    reg = sim[mask].sum()
    loss = sse / float(B * D) + ALPHA * reg
    return np.asarray(np.float32(loss))


def kernel(x_batch: np.ndarray, y_batch: np.ndarray, conv_w: np.ndarray) -> np.ndarray:
    nc = _get_prog()
    A = np.ascontiguousarray(conv_w.reshape(M, G))
    in_maps = []
    for c in range(N_CORES):
        in_maps.append({
            "xs": np.ascontiguousarray(x_batch[c * ROWS:(c + 1) * ROWS]),
            "ys": np.ascontiguousarray(y_batch[c * ROWS:(c + 1) * ROWS]),
            "aw": np.ascontiguousarray(A[c * MC:(c + 1) * MC]),
        })
    res = run_bass_kernel_spmd(nc, in_maps, core_ids=list(range(N_CORES))).results
    C = np.zeros((G, G), np.float64)
    sse = 0.0
    for r in res:
        c1 = r["c1_part"].astype(np.float64)   # rows 0:128, cols 0:192
        c2 = r["c2_part"].astype(np.float64)   # rows/cols 128:192
        C[:128] += c1
        C[128:, :128] += c1[:, 128:].T         # symmetry
        C[128:, 128:] += c2
        sse += float(r["sse_part"].sum(dtype=np.float64))
    return _epilogue(C, sse)


# revision 4
# speedup vs baseline: 1.6632x; 1.6632x over previous
"""Trainium2 Bass kernel for nn_EnhanceDiversityFeatureExtracition.

Computes  loss = mean((x-y)^2) + ALPHA * diversity_reg(conv_w)
where diversity_reg builds a 64x64 Gram matrix of the F=64 slices
conv_w[:, :, i, :] (each flattened to a 786432-vector), normalizes it to
cosine similarities, and sums the entries with tau < sim <= 1 off the
diagonal.

Distribution (8 NeuronCores, SPMD):
  - x_batch / y_batch sharded on batch dim: 256 rows per core.
  - conv_w viewed as A = conv_w.reshape(262144, 192)  (row m = (o,c),
    col = f*3+k).  gram[i,j] = sum_m sum_k A[m,3i+k]*A[m,3j+k], so A is
    sharded along the 262144-row reduction axis: 32768 rows per core.
  - Each core returns the partial 192x192 C = A_shard^T A_shard (as a
    128x192 + 64x192 pair) and per-partition partial sums of (x-y)^2;
    the host sums partials, extracts gram[i,j] = sum_k C[3i+k,3j+k] and
    applies the tiny 64x64 masked-similarity epilogue.

On-core dataflow (v4, tuned from traces):
  - A streams on the Sync HWDGE ring only: strict FIFO = blocks land in
    program order (a split across both rings showed 13us cross-ring
    skew and head-of-line stalls for the in-order matmul consumer).
    24KB-per-partition descriptors on the big blocks.
  - x/y stream on the Activation HWDGE ring as 1MB [128 x 2048] chunks
    (8KB descriptors), issued up-front: the SDMA engines round-robin
    between the two rings per packet, so the xy stream co-drains at a
    ~25% share (8KB vs 24KB descriptors) across the whole run and both
    rings finish together - no dedicated xy windows that would starve
    the PE past the ~3.4us HAM re-throttle horizon.
  - Per 128-row strip: 2 fp32r matmuls (moving width 256 for the
    full-rate mode; a bf16 variant measured 2.4x slower per strip)
    accumulate C into PSUM across all 256 strips.
  - MSE: DVE computes d = x-y, ACT computes Square(d) with per-chunk
    per-partition accumulate; ACTs are queued after the Activation
    ring's DMA issues so they never stall descriptor generation.
"""

import numpy as np

import concourse.bass as bass
import concourse.mybir as mybir
from concourse import bacc, tile
from concourse.bass_utils import run_bass_kernel_spmd

N_CORES = 8
B, D = 2048, 4096            # x_batch / y_batch
M, G = 262144, 192           # conv_w as (M, G); G = F*KW
F, KW = 64, 3
ROWS = B // N_CORES          # 256 batch rows per core
MC = M // N_CORES            # 32768 reduction rows per core
NSTRIP = MC // 128           # 256 strips of 128 rows per core

# A-block plan (strips per block); small blocks last shrink the tail.
A_PLAN = [32, 32, 32, 32, 32, 32, 32, 16, 10, 6]
assert sum(A_PLAN) == NSTRIP

NXY = 4                      # x/y chunk-pairs per core
XYW = (ROWS * D) // (128 * NXY)   # 2048 floats per partition per chunk

ALPHA = 0.0005
TAU = 0.2

_prog = None


def _build() -> bass.Bass:
    nc = bacc.Bacc(None, target_bir_lowering=False)
    f32 = mybir.dt.float32
    f32r = mybir.dt.float32r

    xs = nc.dram_tensor("xs", [ROWS, D], f32, kind="ExternalInput")
    ys = nc.dram_tensor("ys", [ROWS, D], f32, kind="ExternalInput")
    aw = nc.dram_tensor("aw", [MC, G], f32r, kind="ExternalInput")
    c1_part = nc.dram_tensor("c1_part", [128, G], f32, kind="ExternalOutput")
    c2_part = nc.dram_tensor("c2_part", [F, G], f32, kind="ExternalOutput")
    sse_part = nc.dram_tensor("sse_part", [128, NXY], f32, kind="ExternalOutput")

    RW = 256                 # fp32r full-rate moving width
    PAD = RW - G             # 64 junk floats past each block's last strip
    MAXS = max(A_PLAN)

    with tile.TileContext(nc) as tc:
        with (
            tc.tile_pool(name="apool", bufs=4) as apool,
            tc.tile_pool(name="xpool", bufs=4) as xpool,
            tc.tile_pool(name="ypool", bufs=4) as ypool,
            tc.tile_pool(name="dpool", bufs=2) as dpool,
            tc.tile_pool(name="qpool", bufs=1) as qpool,
            tc.tile_pool(name="opool", bufs=1) as opool,
            tc.tile_pool(name="psum", bufs=1, space=bass.MemorySpace.PSUM) as psum,
        ):
            cps1 = psum.tile([128, RW], f32, tag="cps1")
            cps2 = psum.tile([F, RW], f32, tag="cps2")
            acc = opool.tile([128, NXY], f32)

            xv = xs[:].rearrange("(p t) d -> p (t d)", p=128)
            yv = ys[:].rearrange("(p t) d -> p (t d)", p=128)

            acts = []        # deferred (dtile, chunk-idx) Square-accums

            def emit_xy(c):
                xt = xpool.tile([128, XYW], f32)
                nc.scalar.dma_start(xt[:], xv[:, c * XYW:(c + 1) * XYW])
                yt = ypool.tile([128, XYW], f32)
                nc.scalar.dma_start(yt[:], yv[:, c * XYW:(c + 1) * XYW])
                dtile = dpool.tile([128, XYW], f32)
                nc.vector.tensor_sub(dtile[:], xt[:], yt[:])
                acts.append((dtile, c))

            ti = 0           # global strip counter
            row0 = 0
            xy_next = 0
            for bi, ns in enumerate(A_PLAN):
                at = apool.tile([128, MAXS * G + PAD], f32r)
                src = aw[row0:row0 + 128 * ns].rearrange(
                    "(p t) g -> p (t g)", p=128)
                nc.sync.dma_start(at[:, :ns * G], src)
                nc.gpsimd.memset(
                    at[:, ns * G:ns * G + PAD].bitcast(f32), 0.0)
                row0 += 128 * ns

                for t in range(ns):
                    rhs = at[:, t * G:t * G + RW]
                    w1 = at[:, t * G:t * G + 128]
                    w2 = at[:, t * G + 128:t * G + G]
                    nc.tensor.matmul(
                        cps1[:], w1, rhs,
                        start=(ti == 0), stop=(ti == NSTRIP - 1),
                    )
                    nc.tensor.matmul(
                        cps2[:], w2, rhs,
                        start=(ti == 0), stop=(ti == NSTRIP - 1),
                    )
                    ti += 1

                # xy chunks issue early; they co-drain on the other ring
                if bi in (0, 1, 2, 3) and xy_next < NXY:
                    emit_xy(xy_next)
                    xy_next += 1
            while xy_next < NXY:
                emit_xy(xy_next)
                xy_next += 1

            # C partials out (ready once the last A block's matmuls stop)
            o1 = opool.tile([128, G], f32, tag="o1")
            nc.vector.tensor_copy(o1[:], cps1[:, :G])
            nc.sync.dma_start(c1_part[:], o1[:])
            o2 = opool.tile([F, G], f32, tag="o2")
            nc.vector.tensor_copy(o2[:], cps2[:, :G])
            nc.sync.dma_start(c2_part[:], o2[:])

            # deferred MSE squares: queued on ACT after all its DMA issues
            for dtile, c in acts:
                qtile = qpool.tile([128, XYW], f32)
                nc.scalar.activation(
                    qtile[:], dtile[:],
                    mybir.ActivationFunctionType.Square,
                    accum_out=acc[:, c:c + 1],
                )
            nc.scalar.dma_start(sse_part[:], acc[:])

    nc.finalize()
    return nc


def _get_prog() -> bass.Bass:
    global _prog
    if _prog is None:
        _prog = _build()
    return _prog


def _epilogue(C: np.ndarray, sse: float) -> np.ndarray:
    # gram[i,j] = sum_k C[3i+k, 3j+k]
    gram = np.einsum("ikjl,kl->ij", C.reshape(F, KW, F, KW), np.eye(KW))
    norms = np.sqrt(np.diag(gram))
    sim = gram / np.outer(norms, norms)
    mask = (sim > TAU) & (sim <= 1.0) & (~np.eye(F, dtype=bool))
    reg = sim[mask].sum()
    loss = sse / float(B * D) + ALPHA * reg
    return np.asarray(np.float32(loss))


def kernel(x_batch: np.ndarray, y_batch: np.ndarray, conv_w: np.ndarray) -> np.ndarray:
    nc = _get_prog()
    A = np.ascontiguousarray(conv_w.reshape(M, G))
    in_maps = []
    for c in range(N_CORES):
        in_maps.append({
            "xs": np.ascontiguousarray(x_batch[c * ROWS:(c + 1) * ROWS]),
            "ys": np.ascontiguousarray(y_batch[c * ROWS:(c + 1) * ROWS]),
            "aw": np.ascontiguousarray(A[c * MC:(c + 1) * MC]),
        })
    res = run_bass_kernel_spmd(nc, in_maps, core_ids=list(range(N_CORES))).results
    C = np.zeros((G, G), np.float64)
    sse = 0.0
    for r in res:
        C[:128] += r["c1_part"].astype(np.float64)
        C[128:] += r["c2_part"].astype(np.float64)
        sse += float(r["sse_part"].sum(dtype=np.float64))
    return _epilogue(C, sse)
